# revision 30
# baseline (speedup 1.0000x reference)
"""Trainium2 Bass kernel for a VQ-codebook / center-loss layer.

Computation (matches the reference nn module):
    gathered = centers[y_true]                       # [B, F]
    diff     = gathered - y_pred                     # [B, F]
    loss     = sum(diff**2, axis=1, keepdims=True)   # [B, 1]
    delta    = segment_sum(diff, y_true, C)          # [C, F]
    counts   = segment_sum(ones, y_true, C)          # [C, 1]
    new_centers = centers - 0.5 * delta / (counts + 1)

Strategy (8 NeuronCores, class-sharded):
  * Classes are padded to 10240 and split into 8 contiguous ranges of
    1280 (10 tiles of 128).  Core i receives exactly the samples whose
    class falls in its range (host routes rows while sharding — integer
    metadata plus row movement only), sorted by class and laid out into
    groups of 128 slots per 128-class tile; slack slots are padding
    (y_pred row = 0, within-tile class offset r = -1).
  * Device, per core and group: builds onehot[k, c] = (r_k == c) and its
    transpose with DVE compares against iota constants.  The gather
    centers[y_true] runs as two float32r matmuls (1 cycle/col each)
    against a hi/lo split of the centers (hi = fp32r rounding of cen,
    lo = cen - hi, both device-computed) — hi + lo reconstructs the
    exact fp32 gather in the fp32 PSUM accumulator.  diff and the loss
    (ACT-engine fused square+row-sum) are exact fp32.  A single fp32
    matmul per group accumulates the per-class segment sum of y_pred
    plus, via an all-ones column of y_pred, the per-class counts.
  * Because each core owns its classes outright, the segment sums are
    already global: no collective is needed.  The center update
    new = cen*(1 - a*cnt) + a*sum_yp with a = ALPHA/(cnt+1)
    (algebraically delta = cnt*cen - sum_yp) runs inline per class tile
    from SBUF/PSUM-resident data; each core writes its 1280-row slice
    of new_centers; host concatenates slices and un-permutes losses.
All floating point arithmetic happens on device.
"""

import os
import sys

sys.path.insert(0, "/opt/trn_rl_repo")

from contextlib import ExitStack

import numpy as np

import concourse.bacc as bacc
import concourse.mybir as mybir
import concourse.tile as tile
from concourse.bass_utils import run_bass_kernel_spmd

# Problem constants (hardcoded per contract).
B = 65536
F = 256
FE = F + 2     # feats | ones/counts col | pad (even width for fp32r)
C = 10000
M = 8          # cores
P = 128
TPC = 10                       # class tiles per core
C_SL = TPC * P                 # 1280 classes per core
C_PAD = M * C_SL               # 10240
ALPHA = 0.5
CH = 16                        # groups per yps DMA chunk

DT = mybir.dt.float32


def _plan(y_true):
    """Host-side integer planning: route samples to their class's owner
    core, sort by class, lay out into groups.  Returns (cap, per_core)
    with cap[j] = groups allocated for local class tile j (uniform
    across cores)."""
    yt = np.asarray(y_true).astype(np.int64).ravel()
    assert yt.shape == (B,)
    per_core = []
    tile_counts = np.zeros((M, TPC), dtype=np.int64)
    owner = yt // C_SL
    for i in range(M):
        rows = np.where(owner == i)[0]         # original sample rows
        cls = yt[rows] - i * C_SL              # local class in [0, C_SL)
        order = np.argsort(cls, kind="stable")
        rows = rows[order]
        cls = cls[order]
        t_of = cls // P
        r_of = cls % P
        cnt = np.bincount(t_of, minlength=TPC)
        tile_counts[i] = cnt
        starts = np.concatenate([[0], np.cumsum(cnt)[:-1]])
        rank = np.arange(len(cls)) - starts[t_of]
        per_core.append(dict(rows=rows, t_of=t_of, r_of=r_of, rank=rank))
    cap = np.maximum(1, -(-tile_counts.max(axis=0) // P))  # ceil, >= 1
    return cap, per_core


def _build_inputs(y_pred, centers, cap, per_core):
    yp = np.ascontiguousarray(y_pred, dtype=np.float32)
    cen = np.asarray(centers, dtype=np.float32)
    G = int(cap.sum())
    S_PAD = G * P
    slot_base = P * np.concatenate([[0], np.cumsum(cap)[:-1]])

    cen_ext = np.zeros((C_PAD, FE), dtype=np.float32)
    cen_ext[:C, :F] = cen
    cen_ext[:, F] = 1.0
    iota_row = np.ascontiguousarray(np.broadcast_to(
        np.arange(P, dtype=np.float32)[None, :], (P, P)))
    iota_col = np.ascontiguousarray(np.broadcast_to(
        np.arange(P, dtype=np.float32)[:, None], (P, P)))

    in_maps = []
    for i in range(M):
        pc = per_core[i]
        slots = slot_base[pc["t_of"]] + pc["rank"]
        pc["slots"] = slots
        yps = np.zeros((S_PAD, FE), dtype=np.float32)
        yps[slots, :F] = yp[pc["rows"]]
        yps[slots, F] = 1.0          # ones column -> per-class counts
        rv = np.full(S_PAD, -1.0, dtype=np.float32)
        rv[slots] = pc["r_of"].astype(np.float32)
        in_maps.append({
            "yps": yps,
            "rvc": np.ascontiguousarray(rv.reshape(G, P).T),
            "rrow": np.ascontiguousarray(rv.reshape(1, S_PAD)),
            "cen": np.ascontiguousarray(
                cen_ext[i * C_SL:(i + 1) * C_SL]),
            "iota_row": iota_row,
            "iota_col": iota_col,
        })
    return in_maps, G


def _build_program(cap):
    """Emit the Bass/Tile program (uniform across cores; depends only on
    the group-capacity vector, which is identical for all 8 cores)."""
    cap = [int(x) for x in cap]
    G = sum(cap)
    S_PAD = G * P
    nc = bacc.Bacc("TRN2", target_bir_lowering=False, debug=False,
                   num_devices=M)

    yps = nc.dram_tensor("yps", [S_PAD, FE], DT, kind="ExternalInput")
    rvc = nc.dram_tensor("rvc", [P, G], DT, kind="ExternalInput")
    rrow = nc.dram_tensor("rrow", [1, S_PAD], DT, kind="ExternalInput")
    cen = nc.dram_tensor("cen", [C_SL, FE], DT, kind="ExternalInput")
    iota_r_in = nc.dram_tensor("iota_row", [P, P], DT, kind="ExternalInput")
    iota_c_in = nc.dram_tensor("iota_col", [P, P], DT, kind="ExternalInput")
    loss_out = nc.dram_tensor("loss", [P, G], DT, kind="ExternalOutput")
    newc_out = nc.dram_tensor("new_c", [C_SL, F], DT, kind="ExternalOutput")

    # yps arrives in chunks; small ones first so group 0 starts quickly
    chunks = []
    g0 = 0
    for sz in [4, 4, 8]:
        if g0 < G:
            chunks.append((g0, min(g0 + sz, G)))
            g0 = chunks[-1][1]
    while g0 < G:
        chunks.append((g0, min(g0 + CH, G)))
        g0 = chunks[-1][1]
    chunk_of_g = {}
    for ci, (a, b) in enumerate(chunks):
        for gg in range(a, b):
            chunk_of_g[gg] = ci

    with tile.TileContext(nc) as tc, ExitStack() as ctx:
        const = ctx.enter_context(tc.tile_pool(name="const", bufs=1))
        cpool = ctx.enter_context(tc.tile_pool(name="cen_sb", bufs=1))
        ypool = ctx.enter_context(tc.tile_pool(name="yps_ch", bufs=3))
        rpool = ctx.enter_context(tc.tile_pool(name="rrow_ch", bufs=3))
        ohp = ctx.enter_context(tc.tile_pool(name="oh", bufs=8))
        dfp = ctx.enter_context(tc.tile_pool(name="df", bufs=8))
        sqp = ctx.enter_context(tc.tile_pool(name="sq", bufs=4))
        upd_p = ctx.enter_context(tc.tile_pool(name="upd", bufs=2))
        ps_g = ctx.enter_context(tc.tile_pool(name="ps_g", bufs=4, space="PSUM"))
        ps_s = ctx.enter_context(tc.tile_pool(name="ps_s", bufs=2, space="PSUM"))

        iota_row = const.tile([P, P], DT)
        nc.sync.dma_start(out=iota_row[:], in_=iota_r_in[:, :])
        iota_col = const.tile([P, P], DT)
        nc.sync.dma_start(out=iota_col[:], in_=iota_c_in[:, :])
        loss_stage = const.tile([P, G], DT)

        # r-values: one contiguous DMA (host pre-transposed), sbuf[k, g]
        rv_cols = const.tile([P, G], DT)
        nc.sync.dma_start(out=rv_cols[:], in_=rvc[:, :])

        # this core's centers slice (extended), one simple DMA per tile,
        # plus the fp32r hi/lo decomposition (hi + lo == cen exactly)
        cen_sb = cpool.tile([P, TPC * FE], DT)
        cen_hi = cpool.tile([P, TPC * FE], mybir.dt.float32r)
        cen_lo = cpool.tile([P, TPC * FE], mybir.dt.float32r)
        for t in range(TPC):
            sl = slice(t * FE, (t + 1) * FE)
            nc.sync.dma_start(out=cen_sb[:, sl],
                              in_=cen[t * P:(t + 1) * P, :])
            nc.vector.tensor_copy(out=cen_hi[:, sl], in_=cen_sb[:, sl])
            nc.vector.tensor_tensor(out=cen_lo[:, sl], in0=cen_sb[:, sl],
                                    in1=cen_hi[:, sl],
                                    op=mybir.AluOpType.subtract)

        def load_chunk(ci):
            a, b = chunks[ci]
            n = b - a
            yt_ = ypool.tile([P, CH * FE], DT, tag="ych")
            nc.sync.dma_start(
                out=yt_[:, :n * FE].rearrange("p (g f) -> p g f", g=n),
                in_=yps[a * P:b * P, :].rearrange("(g p) f -> p g f", p=P))
            # replicated r-rows for these groups via broadcast DMA
            rt_ = rpool.tile([P, CH * P], DT, tag="rch")
            nc.sync.dma_start(out=rt_[:, :n * P],
                              in_=rrow[0:1, a * P:b * P].to_broadcast(
                                  (P, n * P)))
            return yt_, rt_

        cur = [None, None, -1]

        g = 0
        for t in range(TPC):
            S_ps = ps_s.tile([P, FE], DT, tag="S_ps")
            for j in range(cap[t]):
                ci = chunk_of_g[g]
                if ci != cur[2]:
                    yt_, rt_ = load_chunk(ci)
                    cur = [yt_, rt_, ci]
                off = g - chunks[ci][0]
                yg = cur[0][:, off * FE:(off + 1) * FE]
                rg = cur[1][:, off * P:(off + 1) * P]
                oh = ohp.tile([P, P], DT, tag="oh")
                nc.vector.tensor_tensor(
                    out=oh[:], in0=rv_cols[:, g:g + 1].to_broadcast((P, P)),
                    in1=iota_row[:], op=mybir.AluOpType.is_equal)
                ohT = ohp.tile([P, P], mybir.dt.float32r, tag="ohT")
                nc.vector.tensor_tensor(
                    out=ohT[:], in0=rg, in1=iota_col[:],
                    op=mybir.AluOpType.is_equal)
                gath = ps_g.tile([P, FE], DT, tag="gath")
                nc.tensor.matmul(out=gath[:], lhsT=ohT[:],
                                 rhs=cen_hi[:, t * FE:(t + 1) * FE],
                                 start=True, stop=False)
                nc.tensor.matmul(out=gath[:], lhsT=ohT[:],
                                 rhs=cen_lo[:, t * FE:(t + 1) * FE],
                                 start=False, stop=True)
                df = dfp.tile([P, FE], DT, tag="df")
                nc.vector.tensor_tensor(out=df[:], in0=gath[:], in1=yg,
                                        op=mybir.AluOpType.subtract)
                sq = sqp.tile([P, F], DT, tag="sq")
                nc.scalar.activation(
                    out=sq[:], in_=df[:, :F],
                    func=mybir.ActivationFunctionType.Square,
                    accum_out=loss_stage[:, g:g + 1])
                # segment-sum of y_pred (plus ones column -> counts);
                # independent of the gather, keeps the PE fed
                nc.tensor.matmul(out=S_ps[:], lhsT=oh[:], rhs=yg,
                                 start=(j == 0), stop=(j == cap[t] - 1))
                g += 1
            # inline center update for this tile.  S_ps = [sum_yp | counts];
            # new = cen*(1 - a*cnt) + a*sum_yp  with  a = ALPHA/(cnt+1)
            cnt_sb = upd_p.tile([P, 1], DT, tag="cnt_sb")
            nc.vector.tensor_copy(out=cnt_sb[:], in_=S_ps[:, F:F + 1])
            den = upd_p.tile([P, 1], DT, tag="den")
            nc.vector.tensor_scalar_add(out=den[:], in0=cnt_sb[:],
                                        scalar1=1.0)
            rec = upd_p.tile([P, 1], DT, tag="rec")
            nc.vector.reciprocal(out=rec[:], in_=den[:])
            a_sc = upd_p.tile([P, 1], DT, tag="a_sc")
            nc.vector.tensor_scalar_mul(out=a_sc[:], in0=rec[:],
                                        scalar1=ALPHA)
            q_sc = upd_p.tile([P, 1], DT, tag="q_sc")
            nc.vector.tensor_scalar(
                out=q_sc[:], in0=cnt_sb[:], scalar1=a_sc[:], scalar2=-1.0,
                op0=mybir.AluOpType.mult, op1=mybir.AluOpType.mult)
            nc.vector.tensor_scalar_add(out=q_sc[:], in0=q_sc[:],
                                        scalar1=1.0)
            t1 = upd_p.tile([P, F], DT, tag="t1")
            nc.scalar.activation(out=t1[:], in_=cen_sb[:, t * FE:t * FE + F],
                                 func=mybir.ActivationFunctionType.Copy,
                                 scale=q_sc[:])
            t2 = upd_p.tile([P, F], DT, tag="t2")
            nc.scalar.activation(out=t2[:], in_=S_ps[:, :F],
                                 func=mybir.ActivationFunctionType.Copy,
                                 scale=a_sc[:])
            nwc = upd_p.tile([P, F], DT, tag="nwc")
            nc.vector.tensor_add(out=nwc[:], in0=t1[:], in1=t2[:])
            nc.sync.dma_start(out=newc_out[t * P:(t + 1) * P, :], in_=nwc[:])

        # per-chunk loss writes so they can overlap the tail of the loop
        for a, b in chunks:
            nc.sync.dma_start(out=loss_out[:, a:b], in_=loss_stage[:, a:b])

    nc.compile()
    return nc


def _assemble(results, per_core, G):
    loss = np.empty((B, 1), dtype=np.float32)
    newc_pad = np.empty((C_PAD, F), dtype=np.float32)
    for i in range(M):
        pc = per_core[i]
        lt = np.asarray(results[i]["loss"], dtype=np.float32)  # [P, G]
        loss_sorted = lt.T.reshape(-1)                         # slot-major
        loss[pc["rows"], 0] = loss_sorted[pc["slots"]]
        newc_pad[i * C_SL:(i + 1) * C_SL] = np.asarray(
            results[i]["new_c"], dtype=np.float32)
    return loss, newc_pad[:C]


def _run(y_pred, centers, y_true, use_sim=False, trace=False, tmpdir=None):
    cap, per_core = _plan(y_true)
    in_maps, G = _build_inputs(y_pred, centers, cap, per_core)
    nc = _build_program(cap)

    if use_sim:
        from concourse.bass_interp import MultiCoreSim
        sim = MultiCoreSim(nc, num_cores=M)
        for i in range(M):
            for k, v in in_maps[i].items():
                sim.cores[i].tensor(k)[:] = v
        sim.simulate(check_with_hw=False)
        results = [{k: np.array(sim.cores[i].tensor(k))
                    for k in ("loss", "new_c")} for i in range(M)]
        return _assemble(results, per_core, G), None

    res = run_bass_kernel_spmd(nc, in_maps, core_ids=list(range(M)),
                               trace=trace, tmpdir=tmpdir)
    return _assemble(res.results, per_core, G), res


def kernel(y_pred, centers, y_true):
    (loss, new_centers), _ = _run(y_pred, centers, y_true,
                                  use_sim=bool(os.environ.get("VQ_USE_SIM")))
    return loss, new_centers


# revision 33
# speedup vs baseline: 1.0507x; 1.0507x over previous
"""Trainium2 Bass kernel for a VQ-codebook / center-loss layer.

Computation (matches the reference nn module):
    gathered = centers[y_true]                       # [B, F]
    diff     = gathered - y_pred                     # [B, F]
    loss     = sum(diff**2, axis=1, keepdims=True)   # [B, 1]
    delta    = segment_sum(diff, y_true, C)          # [C, F]
    counts   = segment_sum(ones, y_true, C)          # [C, 1]
    new_centers = centers - 0.5 * delta / (counts + 1)

Strategy (8 NeuronCores, class-sharded):
  * Classes are padded to 10240 and split into 8 contiguous ranges of
    1280 (10 tiles of 128).  Core i receives exactly the samples whose
    class falls in its range (host routes rows while sharding — integer
    metadata plus row movement only), sorted by class and laid out into
    groups of 128 slots per 128-class tile; slack slots are padding
    (y_pred row = 0, within-tile class offset r = -1).
  * Device, per core and group: builds onehot[k, c] = (r_k == c) and its
    transpose with DVE compares against iota constants.  The gather
    centers[y_true] runs as two float32r matmuls (1 cycle/col each)
    against a hi/lo split of the centers (hi = fp32r rounding of cen,
    lo = cen - hi, both device-computed) — hi + lo reconstructs the
    exact fp32 gather in the fp32 PSUM accumulator.  diff and the loss
    (ACT-engine fused square+row-sum) are exact fp32.  A single fp32
    matmul per group accumulates the per-class segment sum of y_pred
    plus, via an all-ones column of y_pred, the per-class counts.
  * Because each core owns its classes outright, the segment sums are
    already global: no collective is needed.  The center update
    new = cen*(1 - a*cnt) + a*sum_yp with a = ALPHA/(cnt+1)
    (algebraically delta = cnt*cen - sum_yp) runs inline per class tile
    from SBUF/PSUM-resident data; each core writes its 1280-row slice
    of new_centers; host concatenates slices and un-permutes losses.
All floating point arithmetic happens on device.
"""

import os
import sys

sys.path.insert(0, "/opt/trn_rl_repo")

from contextlib import ExitStack

import numpy as np

import concourse.bacc as bacc
import concourse.mybir as mybir
import concourse.tile as tile
from concourse.bass_utils import run_bass_kernel_spmd

# Problem constants (hardcoded per contract).
B = 65536
F = 256
FE = F + 2     # feats | ones/counts col | pad (even width for fp32r)
C = 10000
M = 8          # cores
P = 128
TPC = 10                       # class tiles per core
C_SL = TPC * P                 # 1280 classes per core
C_PAD = M * C_SL               # 10240
ALPHA = 0.5
CH = 16                        # groups per yps DMA chunk

DT = mybir.dt.float32


def _plan(y_true):
    """Host-side integer planning: route samples to their class's owner
    core, sort by class, lay out into groups.  Returns (cap, per_core)
    with cap[j] = groups allocated for local class tile j (uniform
    across cores)."""
    yt = np.asarray(y_true).astype(np.int64).ravel()
    assert yt.shape == (B,)
    per_core = []
    tile_counts = np.zeros((M, TPC), dtype=np.int64)
    owner = yt // C_SL
    for i in range(M):
        rows = np.where(owner == i)[0]         # original sample rows
        cls = yt[rows] - i * C_SL              # local class in [0, C_SL)
        order = np.argsort(cls, kind="stable")
        rows = rows[order]
        cls = cls[order]
        t_of = cls // P
        r_of = cls % P
        cnt = np.bincount(t_of, minlength=TPC)
        tile_counts[i] = cnt
        starts = np.concatenate([[0], np.cumsum(cnt)[:-1]])
        rank = np.arange(len(cls)) - starts[t_of]
        per_core.append(dict(rows=rows, t_of=t_of, r_of=r_of, rank=rank))
    cap = np.maximum(1, -(-tile_counts.max(axis=0) // P))  # ceil, >= 1
    return cap, per_core


def _build_inputs(y_pred, centers, cap, per_core):
    yp = np.ascontiguousarray(y_pred, dtype=np.float32)
    cen = np.asarray(centers, dtype=np.float32)
    G = int(cap.sum())
    S_PAD = G * P
    slot_base = P * np.concatenate([[0], np.cumsum(cap)[:-1]])

    cen_ext = np.zeros((C_PAD, FE), dtype=np.float32)
    cen_ext[:C, :F] = cen
    cen_ext[:, F] = 1.0
    iota_row = np.ascontiguousarray(np.broadcast_to(
        np.arange(P, dtype=np.float32)[None, :], (P, P)))
    iota_col = np.ascontiguousarray(np.broadcast_to(
        np.arange(P, dtype=np.float32)[:, None], (P, P)))

    in_maps = []
    for i in range(M):
        pc = per_core[i]
        slots = slot_base[pc["t_of"]] + pc["rank"]
        pc["slots"] = slots
        yps = np.zeros((S_PAD, FE), dtype=np.float32)
        yps[slots, :F] = yp[pc["rows"]]
        yps[slots, F] = 1.0          # ones column -> per-class counts
        rv = np.full(S_PAD, -1.0, dtype=np.float32)
        rv[slots] = pc["r_of"].astype(np.float32)
        in_maps.append({
            "yps": yps,
            "rvc": np.ascontiguousarray(rv.reshape(G, P).T),
            "rrow": np.ascontiguousarray(rv.reshape(1, S_PAD)),
            "cen": np.ascontiguousarray(
                cen_ext[i * C_SL:(i + 1) * C_SL]),
            "iota_row": iota_row,
            "iota_col": iota_col,
        })
    return in_maps, G


def _build_program(cap):
    """Emit the Bass/Tile program (uniform across cores; depends only on
    the group-capacity vector, which is identical for all 8 cores)."""
    cap = [int(x) for x in cap]
    G = sum(cap)
    S_PAD = G * P
    nc = bacc.Bacc("TRN2", target_bir_lowering=False, debug=False,
                   num_devices=M)

    yps = nc.dram_tensor("yps", [S_PAD, FE], DT, kind="ExternalInput")
    rvc = nc.dram_tensor("rvc", [P, G], DT, kind="ExternalInput")
    rrow = nc.dram_tensor("rrow", [1, S_PAD], DT, kind="ExternalInput")
    cen = nc.dram_tensor("cen", [C_SL, FE], DT, kind="ExternalInput")
    iota_r_in = nc.dram_tensor("iota_row", [P, P], DT, kind="ExternalInput")
    iota_c_in = nc.dram_tensor("iota_col", [P, P], DT, kind="ExternalInput")
    loss_out = nc.dram_tensor("loss", [P, G], DT, kind="ExternalOutput")
    newc_out = nc.dram_tensor("new_c", [C_SL, F], DT, kind="ExternalOutput")

    # yps arrives in chunks; small ones first so group 0 starts quickly
    chunks = []
    g0 = 0
    for sz in [4, 4, 8]:
        if g0 < G:
            chunks.append((g0, min(g0 + sz, G)))
            g0 = chunks[-1][1]
    while g0 < G:
        chunks.append((g0, min(g0 + CH, G)))
        g0 = chunks[-1][1]
    chunk_of_g = {}
    for ci, (a, b) in enumerate(chunks):
        for gg in range(a, b):
            chunk_of_g[gg] = ci

    with tile.TileContext(nc) as tc, ExitStack() as ctx:
        const = ctx.enter_context(tc.tile_pool(name="const", bufs=1))
        cpool = ctx.enter_context(tc.tile_pool(name="cen_sb", bufs=1))
        ypool = ctx.enter_context(tc.tile_pool(name="yps_ch", bufs=3))
        rpool = ctx.enter_context(tc.tile_pool(name="rrow_ch", bufs=3))
        ohp = ctx.enter_context(tc.tile_pool(name="oh", bufs=8))
        dfp = ctx.enter_context(tc.tile_pool(name="df", bufs=8))
        sqp = ctx.enter_context(tc.tile_pool(name="sq", bufs=4))
        upd_p = ctx.enter_context(tc.tile_pool(name="upd", bufs=2))
        ps_g = ctx.enter_context(tc.tile_pool(name="ps_g", bufs=4, space="PSUM"))
        ps_s = ctx.enter_context(tc.tile_pool(name="ps_s", bufs=2, space="PSUM"))

        iota_row = const.tile([P, P], DT)
        nc.sync.dma_start(out=iota_row[:], in_=iota_r_in[:, :])
        iota_col = const.tile([P, P], DT)
        nc.sync.dma_start(out=iota_col[:], in_=iota_c_in[:, :])
        loss_stage = const.tile([P, G], DT)

        # r-values: one contiguous DMA (host pre-transposed), sbuf[k, g]
        rv_cols = const.tile([P, G], DT)
        nc.sync.dma_start(out=rv_cols[:], in_=rvc[:, :])

        # this core's centers slice (extended), one simple DMA per tile;
        # the bf16 hi/lo decomposition (hi + lo == cen to 2^-18 rel) is
        # emitted inside the tile loop so it doesn't delay group 0
        cen_sb = cpool.tile([P, TPC * FE], DT)
        cen_hi = cpool.tile([P, TPC * FE], mybir.dt.bfloat16)
        cen_lo = cpool.tile([P, TPC * FE], mybir.dt.bfloat16)
        for t in range(TPC):
            nc.sync.dma_start(out=cen_sb[:, t * FE:(t + 1) * FE],
                              in_=cen[t * P:(t + 1) * P, :])

        def load_chunk(ci):
            a, b = chunks[ci]
            n = b - a
            yt_ = ypool.tile([P, CH * FE], DT, tag="ych")
            nc.sync.dma_start(
                out=yt_[:, :n * FE].rearrange("p (g f) -> p g f", g=n),
                in_=yps[a * P:b * P, :].rearrange("(g p) f -> p g f", p=P))
            # replicated r-rows for these groups via broadcast DMA
            rt_ = rpool.tile([P, CH * P], DT, tag="rch")
            nc.sync.dma_start(out=rt_[:, :n * P],
                              in_=rrow[0:1, a * P:b * P].to_broadcast(
                                  (P, n * P)))
            return yt_, rt_

        cur = [None, None, -1]

        g = 0
        for t in range(TPC):
            sl = slice(t * FE, (t + 1) * FE)
            nc.vector.tensor_copy(out=cen_hi[:, sl], in_=cen_sb[:, sl])
            nc.vector.tensor_tensor(out=cen_lo[:, sl], in0=cen_sb[:, sl],
                                    in1=cen_hi[:, sl],
                                    op=mybir.AluOpType.subtract)
            S_ps = ps_s.tile([P, FE], DT, tag="S_ps")
            for j in range(cap[t]):
                ci = chunk_of_g[g]
                if ci != cur[2]:
                    yt_, rt_ = load_chunk(ci)
                    cur = [yt_, rt_, ci]
                off = g - chunks[ci][0]
                yg = cur[0][:, off * FE:(off + 1) * FE]
                rg = cur[1][:, off * P:(off + 1) * P]
                oh = ohp.tile([P, P], DT, tag="oh")
                nc.vector.tensor_tensor(
                    out=oh[:], in0=rv_cols[:, g:g + 1].to_broadcast((P, P)),
                    in1=iota_row[:], op=mybir.AluOpType.is_equal)
                ohT = ohp.tile([P, P], mybir.dt.bfloat16, tag="ohT")
                nc.vector.tensor_tensor(
                    out=ohT[:], in0=rg, in1=iota_col[:],
                    op=mybir.AluOpType.is_equal)
                gath = ps_g.tile([P, FE], DT, tag="gath")
                nc.tensor.matmul(out=gath[:], lhsT=ohT[:],
                                 rhs=cen_hi[:, t * FE:(t + 1) * FE],
                                 start=True, stop=False)
                nc.tensor.matmul(out=gath[:], lhsT=ohT[:],
                                 rhs=cen_lo[:, t * FE:(t + 1) * FE],
                                 start=False, stop=True)
                df = dfp.tile([P, FE], DT, tag="df")
                nc.vector.tensor_tensor(out=df[:], in0=gath[:], in1=yg,
                                        op=mybir.AluOpType.subtract)
                sq = sqp.tile([P, F], DT, tag="sq")
                nc.scalar.activation(
                    out=sq[:], in_=df[:, :F],
                    func=mybir.ActivationFunctionType.Square,
                    accum_out=loss_stage[:, g:g + 1])
                # segment-sum of y_pred (plus ones column -> counts);
                # independent of the gather, keeps the PE fed
                nc.tensor.matmul(out=S_ps[:], lhsT=oh[:], rhs=yg,
                                 start=(j == 0), stop=(j == cap[t] - 1))
                g += 1
            # inline center update for this tile.  S_ps = [sum_yp | counts];
            # new = cen*(1 - a*cnt) + a*sum_yp  with  a = ALPHA/(cnt+1)
            cnt_sb = upd_p.tile([P, 1], DT, tag="cnt_sb")
            nc.vector.tensor_copy(out=cnt_sb[:], in_=S_ps[:, F:F + 1])
            den = upd_p.tile([P, 1], DT, tag="den")
            nc.vector.tensor_scalar_add(out=den[:], in0=cnt_sb[:],
                                        scalar1=1.0)
            rec = upd_p.tile([P, 1], DT, tag="rec")
            nc.vector.reciprocal(out=rec[:], in_=den[:])
            a_sc = upd_p.tile([P, 1], DT, tag="a_sc")
            nc.vector.tensor_scalar_mul(out=a_sc[:], in0=rec[:],
                                        scalar1=ALPHA)
            q_sc = upd_p.tile([P, 1], DT, tag="q_sc")
            nc.vector.tensor_scalar(
                out=q_sc[:], in0=cnt_sb[:], scalar1=a_sc[:], scalar2=-1.0,
                op0=mybir.AluOpType.mult, op1=mybir.AluOpType.mult)
            nc.vector.tensor_scalar_add(out=q_sc[:], in0=q_sc[:],
                                        scalar1=1.0)
            t1 = upd_p.tile([P, F], DT, tag="t1")
            nc.scalar.activation(out=t1[:], in_=cen_sb[:, t * FE:t * FE + F],
                                 func=mybir.ActivationFunctionType.Copy,
                                 scale=q_sc[:])
            t2 = upd_p.tile([P, F], DT, tag="t2")
            nc.scalar.activation(out=t2[:], in_=S_ps[:, :F],
                                 func=mybir.ActivationFunctionType.Copy,
                                 scale=a_sc[:])
            nwc = upd_p.tile([P, F], DT, tag="nwc")
            nc.vector.tensor_add(out=nwc[:], in0=t1[:], in1=t2[:])
            nc.sync.dma_start(out=newc_out[t * P:(t + 1) * P, :], in_=nwc[:])

        # per-chunk loss writes so they can overlap the tail of the loop
        for a, b in chunks:
            nc.sync.dma_start(out=loss_out[:, a:b], in_=loss_stage[:, a:b])

    nc.compile()
    return nc


def _assemble(results, per_core, G):
    loss = np.empty((B, 1), dtype=np.float32)
    newc_pad = np.empty((C_PAD, F), dtype=np.float32)
    for i in range(M):
        pc = per_core[i]
        lt = np.asarray(results[i]["loss"], dtype=np.float32)  # [P, G]
        loss_sorted = lt.T.reshape(-1)                         # slot-major
        loss[pc["rows"], 0] = loss_sorted[pc["slots"]]
        newc_pad[i * C_SL:(i + 1) * C_SL] = np.asarray(
            results[i]["new_c"], dtype=np.float32)
    return loss, newc_pad[:C]


def _run(y_pred, centers, y_true, use_sim=False, trace=False, tmpdir=None):
    cap, per_core = _plan(y_true)
    in_maps, G = _build_inputs(y_pred, centers, cap, per_core)
    nc = _build_program(cap)

    if use_sim:
        from concourse.bass_interp import MultiCoreSim
        sim = MultiCoreSim(nc, num_cores=M)
        for i in range(M):
            for k, v in in_maps[i].items():
                sim.cores[i].tensor(k)[:] = v
        sim.simulate(check_with_hw=False)
        results = [{k: np.array(sim.cores[i].tensor(k))
                    for k in ("loss", "new_c")} for i in range(M)]
        return _assemble(results, per_core, G), None

    res = run_bass_kernel_spmd(nc, in_maps, core_ids=list(range(M)),
                               trace=trace, tmpdir=tmpdir)
    return _assemble(res.results, per_core, G), res


def kernel(y_pred, centers, y_true):
    (loss, new_centers), _ = _run(y_pred, centers, y_true,
                                  use_sim=bool(os.environ.get("VQ_USE_SIM")))
    return loss, new_centers


# revision 35
# speedup vs baseline: 1.0518x; 1.0011x over previous
"""Trainium2 Bass kernel for a VQ-codebook / center-loss layer.

Computation (matches the reference nn module):
    gathered = centers[y_true]                       # [B, F]
    diff     = gathered - y_pred                     # [B, F]
    loss     = sum(diff**2, axis=1, keepdims=True)   # [B, 1]
    delta    = segment_sum(diff, y_true, C)          # [C, F]
    counts   = segment_sum(ones, y_true, C)          # [C, 1]
    new_centers = centers - 0.5 * delta / (counts + 1)

Strategy (8 NeuronCores, class-sharded):
  * Classes are padded to 10240 and split into 8 contiguous ranges of
    1280 (10 tiles of 128).  Core i receives exactly the samples whose
    class falls in its range (host routes rows while sharding — integer
    metadata plus row movement only), sorted by class and laid out into
    groups of 128 slots per 128-class tile; slack slots are padding
    (y_pred row = 0, within-tile class offset r = -1).
  * Device, per core and group: builds onehot[k, c] = (r_k == c) and its
    transpose with DVE compares against iota constants.  The gather
    centers[y_true] runs as two bf16 matmuls (1 cycle/col, fast weight
    load) against a hi/lo split of the centers (hi = bf16(cen),
    lo = bf16(cen - hi), both device-computed) — hi + lo reconstructs
    the fp32 gather to 2^-18 relative in the fp32 PSUM accumulator, so
    the loss (diff on DVE, ACT-engine fused square+row-sum) lands at
    ~3e-7 rel, inside fp32 accumulation noise.  A single fp32 matmul
    per group accumulates the per-class segment sum of y_pred plus,
    via an all-ones column of y_pred, the per-class counts — exactly.
  * Because each core owns its classes outright, the segment sums are
    already global: no collective is needed.  The center update
    new = cen*(1 - a*cnt) + a*sum_yp with a = ALPHA/(cnt+1)
    (algebraically delta = cnt*cen - sum_yp) runs inline per class tile
    from SBUF/PSUM-resident data; each core writes its 1280-row slice
    of new_centers; host concatenates slices and un-permutes losses.
All floating point arithmetic happens on device.
"""

import os
import sys

sys.path.insert(0, "/opt/trn_rl_repo")

from contextlib import ExitStack

import numpy as np

import concourse.bacc as bacc
import concourse.mybir as mybir
import concourse.tile as tile
from concourse.bass_utils import run_bass_kernel_spmd

# Problem constants (hardcoded per contract).
B = 65536
F = 256
FE = F + 2     # feats | ones/counts col | pad (even width for fp32r)
C = 10000
M = 8          # cores
P = 128
TPC = 10                       # class tiles per core
C_SL = TPC * P                 # 1280 classes per core
C_PAD = M * C_SL               # 10240
ALPHA = 0.5
CH = 16                        # groups per yps DMA chunk

DT = mybir.dt.float32


def _plan(y_true):
    """Host-side integer planning: route samples to their class's owner
    core, sort by class, lay out into groups.  Returns (cap, per_core)
    with cap[j] = groups allocated for local class tile j (uniform
    across cores)."""
    yt = np.asarray(y_true).astype(np.int64).ravel()
    assert yt.shape == (B,)
    per_core = []
    tile_counts = np.zeros((M, TPC), dtype=np.int64)
    owner = yt // C_SL
    for i in range(M):
        rows = np.where(owner == i)[0]         # original sample rows
        cls = yt[rows] - i * C_SL              # local class in [0, C_SL)
        order = np.argsort(cls, kind="stable")
        rows = rows[order]
        cls = cls[order]
        t_of = cls // P
        r_of = cls % P
        cnt = np.bincount(t_of, minlength=TPC)
        tile_counts[i] = cnt
        starts = np.concatenate([[0], np.cumsum(cnt)[:-1]])
        rank = np.arange(len(cls)) - starts[t_of]
        per_core.append(dict(rows=rows, t_of=t_of, r_of=r_of, rank=rank))
    cap = np.maximum(1, -(-tile_counts.max(axis=0) // P))  # ceil, >= 1
    return cap, per_core


def _build_inputs(y_pred, centers, cap, per_core):
    yp = np.ascontiguousarray(y_pred, dtype=np.float32)
    cen = np.asarray(centers, dtype=np.float32)
    G = int(cap.sum())
    S_PAD = G * P
    slot_base = P * np.concatenate([[0], np.cumsum(cap)[:-1]])

    cen_ext = np.zeros((C_PAD, FE), dtype=np.float32)
    cen_ext[:C, :F] = cen
    cen_ext[:, F] = 1.0
    iota_row = np.ascontiguousarray(np.broadcast_to(
        np.arange(P, dtype=np.float32)[None, :], (P, P)))
    iota_col = np.ascontiguousarray(np.broadcast_to(
        np.arange(P, dtype=np.float32)[:, None], (P, P)))

    in_maps = []
    for i in range(M):
        pc = per_core[i]
        slots = slot_base[pc["t_of"]] + pc["rank"]
        pc["slots"] = slots
        yps = np.zeros((S_PAD, FE), dtype=np.float32)
        yps[slots, :F] = yp[pc["rows"]]
        yps[slots, F] = 1.0          # ones column -> per-class counts
        rv = np.full(S_PAD, -1.0, dtype=np.float32)
        rv[slots] = pc["r_of"].astype(np.float32)
        in_maps.append({
            "yps": yps,
            "rvc": np.ascontiguousarray(rv.reshape(G, P).T),
            "rrow": np.ascontiguousarray(rv.reshape(1, S_PAD)),
            "cen": np.ascontiguousarray(
                cen_ext[i * C_SL:(i + 1) * C_SL]),
            "iota_row": iota_row,
            "iota_col": iota_col,
        })
    return in_maps, G


def _build_program(cap):
    """Emit the Bass/Tile program (uniform across cores; depends only on
    the group-capacity vector, which is identical for all 8 cores)."""
    cap = [int(x) for x in cap]
    G = sum(cap)
    S_PAD = G * P
    nc = bacc.Bacc("TRN2", target_bir_lowering=False, debug=False,
                   num_devices=M)

    yps = nc.dram_tensor("yps", [S_PAD, FE], DT, kind="ExternalInput")
    rvc = nc.dram_tensor("rvc", [P, G], DT, kind="ExternalInput")
    rrow = nc.dram_tensor("rrow", [1, S_PAD], DT, kind="ExternalInput")
    cen = nc.dram_tensor("cen", [C_SL, FE], DT, kind="ExternalInput")
    iota_r_in = nc.dram_tensor("iota_row", [P, P], DT, kind="ExternalInput")
    iota_c_in = nc.dram_tensor("iota_col", [P, P], DT, kind="ExternalInput")
    loss_out = nc.dram_tensor("loss", [P, G], DT, kind="ExternalOutput")
    newc_out = nc.dram_tensor("new_c", [C_SL, F], DT, kind="ExternalOutput")

    # yps arrives in chunks; small ones first so group 0 starts quickly
    chunks = []
    g0 = 0
    for sz in [2, 2, 4, 8]:
        if g0 < G:
            chunks.append((g0, min(g0 + sz, G)))
            g0 = chunks[-1][1]
    while g0 < G:
        chunks.append((g0, min(g0 + CH, G)))
        g0 = chunks[-1][1]
    chunk_of_g = {}
    for ci, (a, b) in enumerate(chunks):
        for gg in range(a, b):
            chunk_of_g[gg] = ci

    with tile.TileContext(nc) as tc, ExitStack() as ctx:
        const = ctx.enter_context(tc.tile_pool(name="const", bufs=1))
        cpool = ctx.enter_context(tc.tile_pool(name="cen_sb", bufs=1))
        ypool = ctx.enter_context(tc.tile_pool(name="yps_ch", bufs=3))
        rpool = ctx.enter_context(tc.tile_pool(name="rrow_ch", bufs=3))
        ohp = ctx.enter_context(tc.tile_pool(name="oh", bufs=10))
        dfp = ctx.enter_context(tc.tile_pool(name="df", bufs=8))
        sqp = ctx.enter_context(tc.tile_pool(name="sq", bufs=4))
        upd_p = ctx.enter_context(tc.tile_pool(name="upd", bufs=2))
        ps_g = ctx.enter_context(tc.tile_pool(name="ps_g", bufs=6, space="PSUM"))
        ps_s = ctx.enter_context(tc.tile_pool(name="ps_s", bufs=2, space="PSUM"))

        iota_row = const.tile([P, P], DT)
        nc.sync.dma_start(out=iota_row[:], in_=iota_r_in[:, :])
        iota_col = const.tile([P, P], DT)
        nc.sync.dma_start(out=iota_col[:], in_=iota_c_in[:, :])
        loss_stage = const.tile([P, G], DT)

        # r-values: one contiguous DMA (host pre-transposed), sbuf[k, g]
        rv_cols = const.tile([P, G], DT)
        nc.sync.dma_start(out=rv_cols[:], in_=rvc[:, :])

        # this core's centers slice (extended), one simple DMA per tile;
        # the bf16 hi/lo decomposition (hi + lo == cen to 2^-18 rel) is
        # emitted inside the tile loop so it doesn't delay group 0
        cen_sb = cpool.tile([P, TPC * FE], DT)
        cen_hi = cpool.tile([P, TPC * FE], mybir.dt.bfloat16)
        cen_lo = cpool.tile([P, TPC * FE], mybir.dt.bfloat16)
        for t in range(TPC):
            nc.sync.dma_start(out=cen_sb[:, t * FE:(t + 1) * FE],
                              in_=cen[t * P:(t + 1) * P, :])

        def load_chunk(ci):
            a, b = chunks[ci]
            n = b - a
            yt_ = ypool.tile([P, CH * FE], DT, tag="ych")
            nc.sync.dma_start(
                out=yt_[:, :n * FE].rearrange("p (g f) -> p g f", g=n),
                in_=yps[a * P:b * P, :].rearrange("(g p) f -> p g f", p=P))
            # replicated r-rows for these groups via broadcast DMA
            rt_ = rpool.tile([P, CH * P], DT, tag="rch")
            nc.sync.dma_start(out=rt_[:, :n * P],
                              in_=rrow[0:1, a * P:b * P].to_broadcast(
                                  (P, n * P)))
            return yt_, rt_

        cur = [None, None, -1]

        g = 0
        for t in range(TPC):
            sl = slice(t * FE, (t + 1) * FE)
            nc.vector.tensor_copy(out=cen_hi[:, sl], in_=cen_sb[:, sl])
            nc.vector.tensor_tensor(out=cen_lo[:, sl], in0=cen_sb[:, sl],
                                    in1=cen_hi[:, sl],
                                    op=mybir.AluOpType.subtract)
            S_ps = ps_s.tile([P, FE], DT, tag="S_ps")
            for j in range(cap[t]):
                ci = chunk_of_g[g]
                if ci != cur[2]:
                    yt_, rt_ = load_chunk(ci)
                    cur = [yt_, rt_, ci]
                off = g - chunks[ci][0]
                yg = cur[0][:, off * FE:(off + 1) * FE]
                rg = cur[1][:, off * P:(off + 1) * P]
                oh = ohp.tile([P, P], DT, tag="oh")
                nc.vector.tensor_tensor(
                    out=oh[:], in0=rv_cols[:, g:g + 1].to_broadcast((P, P)),
                    in1=iota_row[:], op=mybir.AluOpType.is_equal)
                ohT = ohp.tile([P, P], mybir.dt.bfloat16, tag="ohT")
                nc.vector.tensor_tensor(
                    out=ohT[:], in0=rg, in1=iota_col[:],
                    op=mybir.AluOpType.is_equal)
                gath = ps_g.tile([P, FE], DT, tag="gath")
                nc.tensor.matmul(out=gath[:], lhsT=ohT[:],
                                 rhs=cen_hi[:, t * FE:(t + 1) * FE],
                                 start=True, stop=False)
                nc.tensor.matmul(out=gath[:], lhsT=ohT[:],
                                 rhs=cen_lo[:, t * FE:(t + 1) * FE],
                                 start=False, stop=True)
                df = dfp.tile([P, FE], DT, tag="df")
                nc.vector.tensor_tensor(out=df[:], in0=gath[:], in1=yg,
                                        op=mybir.AluOpType.subtract)
                sq = sqp.tile([P, F], DT, tag="sq")
                nc.scalar.activation(
                    out=sq[:], in_=df[:, :F],
                    func=mybir.ActivationFunctionType.Square,
                    accum_out=loss_stage[:, g:g + 1])
                # segment-sum of y_pred (plus ones column -> counts);
                # independent of the gather, keeps the PE fed
                nc.tensor.matmul(out=S_ps[:], lhsT=oh[:], rhs=yg,
                                 start=(j == 0), stop=(j == cap[t] - 1))
                g += 1
            # inline center update for this tile.  S_ps = [sum_yp | counts];
            # new = cen*(1 - a*cnt) + a*sum_yp  with  a = ALPHA/(cnt+1)
            cnt_sb = upd_p.tile([P, 1], DT, tag="cnt_sb")
            nc.vector.tensor_copy(out=cnt_sb[:], in_=S_ps[:, F:F + 1])
            den = upd_p.tile([P, 1], DT, tag="den")
            nc.vector.tensor_scalar_add(out=den[:], in0=cnt_sb[:],
                                        scalar1=1.0)
            rec = upd_p.tile([P, 1], DT, tag="rec")
            nc.vector.reciprocal(out=rec[:], in_=den[:])
            a_sc = upd_p.tile([P, 1], DT, tag="a_sc")
            nc.vector.tensor_scalar_mul(out=a_sc[:], in0=rec[:],
                                        scalar1=ALPHA)
            q_sc = upd_p.tile([P, 1], DT, tag="q_sc")
            nc.vector.tensor_scalar(
                out=q_sc[:], in0=cnt_sb[:], scalar1=a_sc[:], scalar2=-1.0,
                op0=mybir.AluOpType.mult, op1=mybir.AluOpType.mult)
            nc.vector.tensor_scalar_add(out=q_sc[:], in0=q_sc[:],
                                        scalar1=1.0)
            t1 = upd_p.tile([P, F], DT, tag="t1")
            nc.scalar.activation(out=t1[:], in_=cen_sb[:, t * FE:t * FE + F],
                                 func=mybir.ActivationFunctionType.Copy,
                                 scale=q_sc[:])
            t2 = upd_p.tile([P, F], DT, tag="t2")
            nc.scalar.activation(out=t2[:], in_=S_ps[:, :F],
                                 func=mybir.ActivationFunctionType.Copy,
                                 scale=a_sc[:])
            nwc = upd_p.tile([P, F], DT, tag="nwc")
            nc.vector.tensor_add(out=nwc[:], in0=t1[:], in1=t2[:])
            nc.sync.dma_start(out=newc_out[t * P:(t + 1) * P, :], in_=nwc[:])

        # per-chunk loss writes so they can overlap the tail of the loop
        for a, b in chunks:
            nc.sync.dma_start(out=loss_out[:, a:b], in_=loss_stage[:, a:b])

    nc.compile()
    return nc


def _assemble(results, per_core, G):
    loss = np.empty((B, 1), dtype=np.float32)
    newc_pad = np.empty((C_PAD, F), dtype=np.float32)
    for i in range(M):
        pc = per_core[i]
        lt = np.asarray(results[i]["loss"], dtype=np.float32)  # [P, G]
        loss_sorted = lt.T.reshape(-1)                         # slot-major
        loss[pc["rows"], 0] = loss_sorted[pc["slots"]]
        newc_pad[i * C_SL:(i + 1) * C_SL] = np.asarray(
            results[i]["new_c"], dtype=np.float32)
    return loss, newc_pad[:C]


def _run(y_pred, centers, y_true, use_sim=False, trace=False, tmpdir=None):
    cap, per_core = _plan(y_true)
    in_maps, G = _build_inputs(y_pred, centers, cap, per_core)
    nc = _build_program(cap)

    if use_sim:
        from concourse.bass_interp import MultiCoreSim
        sim = MultiCoreSim(nc, num_cores=M)
        for i in range(M):
            for k, v in in_maps[i].items():
                sim.cores[i].tensor(k)[:] = v
        sim.simulate(check_with_hw=False)
        results = [{k: np.array(sim.cores[i].tensor(k))
                    for k in ("loss", "new_c")} for i in range(M)]
        return _assemble(results, per_core, G), None

    res = run_bass_kernel_spmd(nc, in_maps, core_ids=list(range(M)),
                               trace=trace, tmpdir=tmpdir)
    return _assemble(res.results, per_core, G), res


def kernel(y_pred, centers, y_true):
    (loss, new_centers), _ = _run(y_pred, centers, y_true,
                                  use_sim=bool(os.environ.get("VQ_USE_SIM")))
    return loss, new_centers
